# revision 3
# baseline (speedup 1.0000x reference)
"""ColumnParallelLinearWithMoE Trainium2 kernel.

Expert-parallel: expert e -> NeuronCore e. Each core computes
    y_e [8192, 512] = x_e [8192, 1024] @ W_e.T [1024, 512] (+ b_e)
where x_e = input_[idx_list[e]] flattened over (per, seq).

Routing gather/scatter and all layout shuffles happen on the host; the
device does the dense matmul in bf16 (fp32 PSUM accumulation) and stores
bf16. The bias is zero in this module (zero-initialized); if a nonzero
bias ever shows up it is applied on the host during unshard, so the
device pipeline is matmul -> DVE copy (fp32->bf16) -> store.

Measured HW model (v4 trace, 133.0us):
- exec window = [first user inst ~9.94us (DGE ring-init memsets), end of
  trace]. The tail after the last store's completion is ~8.0us of fixed
  framework work (drain + 2 barriers + RANGE_CLEAR + a 254-semaphore
  postamble storm, ~55 EVENT_SEMAPHOREs per engine, Tensor slowest at
  ~115ns each). Not kernel-proportional; don't fight it.
- First DMA instruction ~10.66us (after Tile entry barrier); first byte
  lands ~2.9us later (HWDGE descriptor-gen + SDMA/HBM pipe fill); early
  transfers run ~150-300 GB/s (ramping), steady 2MB/16KB-desc transfers
  ~345 GB/s.
- Warm matmul cadence 220.4ns for [128x128]x[128x512] bf16 (ideal
  213.3 + NX dispatch; each mm carries an LDWEIGHTS).
- v4's flaw: j-outer first sweep needs w[0:4]+x0c0 = 768KB before the
  first real matmul -> stream started 18.9us.

v5: k-outer first superblock. First consumable unit is w_k0 (128KB, sync
ring) + x_k0 (256KB, scalar ring, which is idle this early) = 384KB
split over two DGE streams -> real stream starts ~16.3us. The x layout
for s=0 is k-major (xs[p,0,k*1024+j*128+t]) so each k-group of 8
matmuls consumes one contiguous 256KB chunk; s>=1 stay j-major with
single 2MB loads. N_WARM garbage matmuls bridge PE activity from
~11.3us so HAM un-throttles (1.2->2.4GHz) before the real stream.
Tail: last superblock stores per-j alternating rings; the final j is
evicted+stored in two [128,256] halves on both rings so the last
store's data time is minimal (the ~2us DGE fixed latency dominates).

Device-side layouts (p = SBUF partition = low 7 bits of the d index,
t = token-in-tile, j = token tile, k = contraction tile, s = super):
  xs [128, NSUP, TPS*KT*128]:
      s == 0: xs[p, 0, k*1024 + j*128 + t] = x[j*128 + t, k*128 + p]
      s >= 1: xs[p, s, j*1024 + k*128 + t] = x[s*1024 + j*128 + t, k*128 + p]
  ws [128, KT*OPP]:           ws[p, k*512 + c] = W.T[d, c]
  ys [128, NSUP, TPS*OPP]:    ys[p, s, j*512 + c] = y[s*1024 + j*128 + p, c]
"""

import sys

if "/opt/trn_rl_repo" not in sys.path:
    sys.path.insert(0, "/opt/trn_rl_repo")

import numpy as np

VARIANT = "v5"

# Problem constants (hardcoded per harness contract).
E = 8
BS = 64
S = 1024
D = 1024
OPP = 512
P = 128
TOK = (BS // E) * S  # 8192 tokens per expert
KT = D // P          # 8 contraction tiles
TW = 1024            # token-superblock width staged in SBUF
NSUP = TOK // TW     # 8 superblocks
TPS = TW // P        # 8 token tiles (of 128) per superblock
CH = KT * P          # 1024 elements per chunk per partition (j- or k-chunk)

N_WARM = 14          # warmup matmuls bridging the initial DMA window

_programs: dict[str, tuple] = {}


def _build():
    import concourse.bacc as bacc
    import concourse.tile as tile
    from concourse import mybir
    import ml_dtypes

    mm_dt = mybir.dt.bfloat16
    np_in = ml_dtypes.bfloat16

    nc = bacc.Bacc(None, target_bir_lowering=False, debug=False)

    xs = nc.dram_tensor("xs", [P, NSUP, TPS * KT * P], mm_dt, kind="ExternalInput")
    ws = nc.dram_tensor("ws", [P, KT * OPP], mm_dt, kind="ExternalInput")
    ys = nc.dram_tensor("ys", [P, NSUP, TPS * OPP], mm_dt, kind="ExternalOutput")

    with tile.TileContext(nc) as tc:
        with (
            tc.tile_pool(name="wpool", bufs=1) as wpool,
            tc.tile_pool(name="xpool", bufs=4) as xpool,
            tc.tile_pool(name="opool", bufs=3) as opool,
            tc.tile_pool(name="pspool", bufs=8, space="PSUM") as pspool,
        ):
            x0_sb = xpool.tile([P, TPS * KT * P], mm_dt, tag="x")
            w_sb = wpool.tile([P, KT * OPP], mm_dt)
            # Head loads, two parallel DGE streams:
            #   sync ring:   w k-chunks (128KB each), then the s>=1 2MB
            #                superblocks queue behind.
            #   scalar ring: x0 k-chunks (256KB each); stores queue behind.
            # Both rings interleave on the shared SDMA engines, so the
            # first consumable set (w_k0 + x_k0) is only 384KB deep.
            for k in range(KT):
                nc.sync.dma_start(
                    out=w_sb[:, k * OPP : (k + 1) * OPP],
                    in_=ws[:, k * OPP : (k + 1) * OPP],
                )
            for k in range(KT):
                nc.scalar.dma_start(
                    out=x0_sb[:, k * CH : (k + 1) * CH],
                    in_=xs[:, 0, k * CH : (k + 1) * CH],
                )

            # PE warmup on a zeroed tile: keeps the PE busy from ~11.3us
            # (memset done ~11.2) so HAM un-throttles by ~14.7us, before
            # the real stream starts ~16.3us (idle gap < the ~5us HAM
            # re-throttle window).
            warm_src = wpool.tile([P, OPP], mm_dt, tag="warm")
            nc.gpsimd.memset(warm_src[:], 0.0)
            warm_ps = pspool.tile([P, OPP], mybir.dt.float32, tag="ps")
            for _ in range(N_WARM):
                nc.tensor.matmul(
                    warm_ps[:], warm_src[:, :P], warm_src[:], start=True, stop=True
                )

            # Superblock 0: k-outer so each k-group of 8 matmuls needs only
            # (w_k, x0_k) = 384KB; all 8 PSUM banks accumulate in parallel.
            ps0 = [
                pspool.tile([P, OPP], mybir.dt.float32, tag="ps", name=f"ps0_{j}")
                for j in range(TPS)
            ]
            for k in range(KT):
                for j in range(TPS):
                    nc.tensor.matmul(
                        ps0[j][:],
                        x0_sb[:, k * CH + j * P : k * CH + (j + 1) * P],
                        w_sb[:, k * OPP : (k + 1) * OPP],
                        start=(k == 0),
                        stop=(k == KT - 1),
                    )
            o0_sb = opool.tile([P, TPS * OPP], mm_dt, tag="o")
            for j in range(TPS):
                nc.vector.tensor_copy(o0_sb[:, j * OPP : (j + 1) * OPP], ps0[j][:])
            nc.scalar.dma_start(out=ys[:, 0, :], in_=o0_sb[:])

            # Superblocks 1..7: j-outer steady state.
            for s in range(1, NSUP):
                x_sb = xpool.tile([P, TPS * KT * P], mm_dt, tag="x")
                nc.sync.dma_start(out=x_sb[:], in_=xs[:, s, :])
                o_sb = opool.tile([P, TPS * OPP], mm_dt, tag="o")
                last_s = s == NSUP - 1
                for j in range(TPS):
                    ps = pspool.tile([P, OPP], mybir.dt.float32, tag="ps")
                    for k in range(KT):
                        nc.tensor.matmul(
                            ps[:],
                            x_sb[:, j * CH + k * P : j * CH + (k + 1) * P],
                            w_sb[:, k * OPP : (k + 1) * OPP],
                            start=(k == 0),
                            stop=(k == KT - 1),
                        )
                    if not last_s:
                        nc.vector.tensor_copy(
                            o_sb[:, j * OPP : (j + 1) * OPP], ps[:]
                        )
                    elif j < TPS - 1:
                        # Fine-grained tail: store each token tile as soon
                        # as it is evicted, alternating rings (both idle).
                        nc.vector.tensor_copy(
                            o_sb[:, j * OPP : (j + 1) * OPP], ps[:]
                        )
                        eng = nc.scalar if (j % 2 == 0) else nc.sync
                        eng.dma_start(
                            out=ys[:, s, j * OPP : (j + 1) * OPP],
                            in_=o_sb[:, j * OPP : (j + 1) * OPP],
                        )
                    else:
                        # Final tile in two halves on both rings so the
                        # last store is only 64KB of data behind the last
                        # matmul + half-CAST.
                        H = OPP // 2
                        lo = j * OPP
                        nc.vector.tensor_copy(
                            o_sb[:, lo : lo + H], ps[:, 0:H]
                        )
                        nc.sync.dma_start(
                            out=ys[:, s, lo : lo + H], in_=o_sb[:, lo : lo + H]
                        )
                        nc.vector.tensor_copy(
                            o_sb[:, lo + H : lo + OPP], ps[:, H:OPP]
                        )
                        nc.scalar.dma_start(
                            out=ys[:, s, lo + H : lo + OPP],
                            in_=o_sb[:, lo + H : lo + OPP],
                        )
                if not last_s:
                    nc.scalar.dma_start(out=ys[:, s, :], in_=o_sb[:])

    nc.compile()
    return nc, np_in


def _get_program():
    if VARIANT not in _programs:
        _programs[VARIANT] = _build()
    return _programs[VARIANT]


def kernel(input_, idx_list, W, b, **_ignored):
    from concourse.bass_utils import run_bass_kernel_spmd

    input_ = np.asarray(input_)
    idx = np.asarray(idx_list).astype(np.int64)
    W = np.asarray(W, dtype=np.float32)
    b = np.asarray(b, dtype=np.float32)

    nc, np_in = _get_program()

    in_maps = []
    for e in range(E):
        xg = input_[idx[e]].reshape(TOK, D)
        # base[s, j, t, k, p] = x[s*1024 + j*128 + t, k*128 + p]
        base = xg.reshape(NSUP, TPS, P, KT, P)
        # s >= 1 j-major: xs[p, s, j*1024 + k*128 + t]
        xhost = np.ascontiguousarray(base.transpose(4, 0, 1, 3, 2)).reshape(
            P, NSUP, TPS * KT * P
        )
        # s == 0 k-major: xs[p, 0, k*1024 + j*128 + t]
        xhost[:, 0, :] = (
            base[0].transpose(3, 2, 0, 1).reshape(P, TPS * KT * P)
        )
        # ws[p, k*512 + c] = W[c, k*128 + p]
        whost = np.ascontiguousarray(
            W[e].reshape(OPP, KT, P).transpose(2, 1, 0)
        ).reshape(P, KT * OPP)
        in_maps.append(
            {"xs": xhost.astype(np_in), "ws": whost.astype(np_in)}
        )

    res = run_bass_kernel_spmd(nc, in_maps, core_ids=list(range(E)))

    out = np.zeros((BS, S, E * OPP), dtype=input_.dtype)
    for e in range(E):
        yd = np.asarray(res.results[e]["ys"]).astype(input_.dtype)
        # ys[p, s, j*512 + c] -> y[s*1024 + j*128 + p, c]
        ye = yd.reshape(P, NSUP, TPS, OPP).transpose(1, 2, 0, 3).reshape(
            BS // E, S, OPP
        )
        if b[e].any():
            ye = ye + b[e][None, None, :]
        out[idx[e], :, e * OPP : (e + 1) * OPP] = ye
    return out


# revision 4
# speedup vs baseline: 1.0076x; 1.0076x over previous
"""ColumnParallelLinearWithMoE Trainium2 kernel.

Expert-parallel: expert e -> NeuronCore e. Each core computes
    y_e [8192, 512] = x_e [8192, 1024] @ W_e.T [1024, 512] (+ b_e)
where x_e = input_[idx_list[e]] flattened over (per, seq).

Routing gather/scatter and all layout shuffles happen on the host; the
device does the dense matmul in bf16 (fp32 PSUM accumulation) and stores
bf16. The bias is zero in this module (zero-initialized); if a nonzero
bias ever shows up it is applied on the host during unshard, so the
device pipeline is matmul -> DVE copy (fp32->bf16) -> store.

Measured HW model (v4 trace, 133.0us):
- exec window = [first user inst ~9.94us (DGE ring-init memsets), end of
  trace]. The tail after the last store's completion is ~8.0us of fixed
  framework work (drain + 2 barriers + RANGE_CLEAR + a 254-semaphore
  postamble storm, ~55 EVENT_SEMAPHOREs per engine, Tensor slowest at
  ~115ns each). Not kernel-proportional; don't fight it.
- First DMA instruction ~10.66us (after Tile entry barrier); first byte
  lands ~2.9us later (HWDGE descriptor-gen + SDMA/HBM pipe fill); early
  transfers run ~150-300 GB/s (ramping), steady 2MB/16KB-desc transfers
  ~345 GB/s.
- Warm matmul cadence 220.4ns for [128x128]x[128x512] bf16 (ideal
  213.3 + NX dispatch; each mm carries an LDWEIGHTS).
- v4's flaw: j-outer first sweep needs w[0:4]+x0c0 = 768KB before the
  first real matmul -> stream started 18.9us.

v5: k-outer first superblock. First consumable unit is w_k0 (128KB, sync
ring) + x_k0 (256KB, scalar ring, which is idle this early) = 384KB
split over two DGE streams -> real stream starts ~16.3us. The x layout
for s=0 is k-major (xs[p,0,k*1024+j*128+t]) so each k-group of 8
matmuls consumes one contiguous 256KB chunk; s>=1 stay j-major with
single 2MB loads. N_WARM garbage matmuls bridge PE activity from
~11.3us so HAM un-throttles (1.2->2.4GHz) before the real stream.
Tail: last superblock stores per-j alternating rings; the final j is
evicted+stored in two [128,256] halves on both rings so the last
store's data time is minimal (the ~2us DGE fixed latency dominates).

Device-side layouts (p = SBUF partition = low 7 bits of the d index,
t = token-in-tile, j = token tile, k = contraction tile, s = super):
  xs [128, NSUP, TPS*KT*128]:
      s == 0: xs[p, 0, k*1024 + j*128 + t] = x[j*128 + t, k*128 + p]
      s >= 1: xs[p, s, j*1024 + k*128 + t] = x[s*1024 + j*128 + t, k*128 + p]
  ws [128, KT*OPP]:           ws[p, k*512 + c] = W.T[d, c]
  ys [128, NSUP, TPS*OPP]:    ys[p, s, j*512 + c] = y[s*1024 + j*128 + p, c]
"""

import sys

if "/opt/trn_rl_repo" not in sys.path:
    sys.path.insert(0, "/opt/trn_rl_repo")

import numpy as np

VARIANT = "v6"

# Problem constants (hardcoded per harness contract).
E = 8
BS = 64
S = 1024
D = 1024
OPP = 512
P = 128
TOK = (BS // E) * S  # 8192 tokens per expert
KT = D // P          # 8 contraction tiles
TW = 1024            # token-superblock width staged in SBUF
NSUP = TOK // TW     # 8 superblocks
TPS = TW // P        # 8 token tiles (of 128) per superblock
CH = KT * P          # 1024 elements per chunk per partition (j- or k-chunk)

N_WARM = 12          # warmup matmuls bridging the initial DMA window

_programs: dict[str, tuple] = {}


def _build():
    import concourse.bacc as bacc
    import concourse.tile as tile
    from concourse import mybir
    import ml_dtypes

    mm_dt = mybir.dt.bfloat16
    np_in = ml_dtypes.bfloat16

    nc = bacc.Bacc(None, target_bir_lowering=False, debug=False)

    xs = nc.dram_tensor("xs", [P, NSUP, TPS * KT * P], mm_dt, kind="ExternalInput")
    ws = nc.dram_tensor("ws", [P, KT * OPP], mm_dt, kind="ExternalInput")
    ys = nc.dram_tensor("ys", [P, NSUP, TPS * OPP], mm_dt, kind="ExternalOutput")

    with tile.TileContext(nc) as tc:
        with (
            tc.tile_pool(name="wpool", bufs=1) as wpool,
            tc.tile_pool(name="xpool", bufs=4) as xpool,
            tc.tile_pool(name="opool", bufs=3) as opool,
            tc.tile_pool(name="pspool", bufs=8, space="PSUM") as pspool,
        ):
            x0_sb = xpool.tile([P, TPS * KT * P], mm_dt, tag="x")
            w_sb = wpool.tile([P, KT * OPP], mm_dt)
            # Head loads, two parallel DGE streams:
            #   sync ring:   w k-chunks (128KB each), then the s>=1 2MB
            #                superblocks queue behind.
            #   scalar ring: x0 k-chunks (256KB each); stores queue behind.
            # Both rings interleave on the shared SDMA engines, so the
            # first consumable set (w_k0 + x_k0) is only 384KB deep.
            nc.sync.dma_start(out=w_sb[:, 0 : 2 * OPP], in_=ws[:, 0 : 2 * OPP])
            nc.sync.dma_start(out=w_sb[:, 2 * OPP :], in_=ws[:, 2 * OPP :])
            for k in range(KT):
                nc.scalar.dma_start(
                    out=x0_sb[:, k * CH : (k + 1) * CH],
                    in_=xs[:, 0, k * CH : (k + 1) * CH],
                )

            # PE warmup on a zeroed tile: keeps the PE busy from ~11.3us
            # (memset done ~11.2) so HAM un-throttles by ~14.7us, before
            # the real stream starts ~16.3us (idle gap < the ~5us HAM
            # re-throttle window).
            warm_src = wpool.tile([P, OPP], mm_dt, tag="warm")
            nc.gpsimd.memset(warm_src[:], 0.0)
            warm_ps = pspool.tile([P, OPP], mybir.dt.float32, tag="ps")
            for _ in range(N_WARM):
                nc.tensor.matmul(
                    warm_ps[:], warm_src[:, :P], warm_src[:], start=True, stop=True
                )

            # Superblock 0: k-outer so each k-group of 8 matmuls needs only
            # (w_k, x0_k) = 384KB; all 8 PSUM banks accumulate in parallel.
            ps0 = [
                pspool.tile([P, OPP], mybir.dt.float32, tag="ps", name=f"ps0_{j}")
                for j in range(TPS)
            ]
            o0_sb = opool.tile([P, TPS * OPP], mm_dt, tag="o")
            for k in range(KT):
                for j in range(TPS):
                    nc.tensor.matmul(
                        ps0[j][:],
                        x0_sb[:, k * CH + j * P : k * CH + (j + 1) * P],
                        w_sb[:, k * OPP : (k + 1) * OPP],
                        start=(k == 0),
                        stop=(k == KT - 1),
                    )
                    if k == KT - 1:
                        # Evict each bank as soon as its accumulation ends;
                        # keeps the DVE waits tight to the stop matmuls.
                        nc.vector.tensor_copy(
                            o0_sb[:, j * OPP : (j + 1) * OPP], ps0[j][:]
                        )
            nc.scalar.dma_start(out=ys[:, 0, :], in_=o0_sb[:])

            # Superblocks 1..7: j-outer steady state.
            for s in range(1, NSUP):
                x_sb = xpool.tile([P, TPS * KT * P], mm_dt, tag="x")
                nc.sync.dma_start(out=x_sb[:], in_=xs[:, s, :])
                o_sb = opool.tile([P, TPS * OPP], mm_dt, tag="o")
                last_s = s == NSUP - 1
                for j in range(TPS):
                    ps = pspool.tile([P, OPP], mybir.dt.float32, tag="ps")
                    for k in range(KT):
                        nc.tensor.matmul(
                            ps[:],
                            x_sb[:, j * CH + k * P : j * CH + (k + 1) * P],
                            w_sb[:, k * OPP : (k + 1) * OPP],
                            start=(k == 0),
                            stop=(k == KT - 1),
                        )
                    nc.vector.tensor_copy(
                        o_sb[:, j * OPP : (j + 1) * OPP], ps[:]
                    )
                    if last_s:
                        # Fine-grained tail: store each token tile as soon
                        # as it is evicted.
                        nc.scalar.dma_start(
                            out=ys[:, s, j * OPP : (j + 1) * OPP],
                            in_=o_sb[:, j * OPP : (j + 1) * OPP],
                        )
                if not last_s:
                    nc.scalar.dma_start(out=ys[:, s, :], in_=o_sb[:])

    nc.compile()
    return nc, np_in


def _get_program():
    if VARIANT not in _programs:
        _programs[VARIANT] = _build()
    return _programs[VARIANT]


def kernel(input_, idx_list, W, b, **_ignored):
    from concourse.bass_utils import run_bass_kernel_spmd

    input_ = np.asarray(input_)
    idx = np.asarray(idx_list).astype(np.int64)
    W = np.asarray(W, dtype=np.float32)
    b = np.asarray(b, dtype=np.float32)

    nc, np_in = _get_program()

    in_maps = []
    for e in range(E):
        xg = input_[idx[e]].reshape(TOK, D)
        # base[s, j, t, k, p] = x[s*1024 + j*128 + t, k*128 + p]
        base = xg.reshape(NSUP, TPS, P, KT, P)
        # s >= 1 j-major: xs[p, s, j*1024 + k*128 + t]
        xhost = np.ascontiguousarray(base.transpose(4, 0, 1, 3, 2)).reshape(
            P, NSUP, TPS * KT * P
        )
        # s == 0 k-major: xs[p, 0, k*1024 + j*128 + t]
        xhost[:, 0, :] = (
            base[0].transpose(3, 2, 0, 1).reshape(P, TPS * KT * P)
        )
        # ws[p, k*512 + c] = W[c, k*128 + p]
        whost = np.ascontiguousarray(
            W[e].reshape(OPP, KT, P).transpose(2, 1, 0)
        ).reshape(P, KT * OPP)
        in_maps.append(
            {"xs": xhost.astype(np_in), "ws": whost.astype(np_in)}
        )

    res = run_bass_kernel_spmd(nc, in_maps, core_ids=list(range(E)))

    out = np.zeros((BS, S, E * OPP), dtype=input_.dtype)
    for e in range(E):
        yd = np.asarray(res.results[e]["ys"]).astype(input_.dtype)
        # ys[p, s, j*512 + c] -> y[s*1024 + j*128 + p, c]
        ye = yd.reshape(P, NSUP, TPS, OPP).transpose(1, 2, 0, 3).reshape(
            BS // E, S, OPP
        )
        if b[e].any():
            ye = ye + b[e][None, None, :]
        out[idx[e], :, e * OPP : (e + 1) * OPP] = ye
    return out


# revision 8
# speedup vs baseline: 1.0712x; 1.0631x over previous
"""ColumnParallelLinearWithMoE Trainium2 kernel.

Expert-parallel: expert e -> NeuronCore e. Each core computes
    y_e [8192, 512] = x_e [8192, 1024] @ W_e.T [1024, 512] (+ b_e)
where x_e = input_[idx_list[e]] flattened over (per, seq).

Routing gather/scatter and all layout shuffles happen on the host; the
device does the dense matmul in bf16 (fp32 PSUM accumulation) and stores
bf16. The bias is zero in this module (zero-initialized); if a nonzero
bias ever shows up it is applied on the host during unshard, so the
device pipeline is matmul -> DVE copy (fp32->bf16) -> store.

Measured HW model (v4 trace, 133.0us):
- exec window = [first user inst ~9.94us (DGE ring-init memsets), end of
  trace]. The tail after the last store's completion is ~8.0us of fixed
  framework work (drain + 2 barriers + RANGE_CLEAR + a 254-semaphore
  postamble storm, ~55 EVENT_SEMAPHOREs per engine, Tensor slowest at
  ~115ns each). Not kernel-proportional; don't fight it.
- First DMA instruction ~10.66us (after Tile entry barrier); first byte
  lands ~2.9us later (HWDGE descriptor-gen + SDMA/HBM pipe fill); early
  transfers run ~150-300 GB/s (ramping), steady 2MB/16KB-desc transfers
  ~345 GB/s.
- Warm matmul cadence 220.4ns for [128x128]x[128x512] bf16 (ideal
  213.3 + NX dispatch; each mm carries an LDWEIGHTS).
- v4's flaw: j-outer first sweep needs w[0:4]+x0c0 = 768KB of serial ring
  data before the first real matmul -> stream started 8.95us after the
  window opened.

Dead ends measured in v5/v6 (do not revisit):
- Splitting loads across the sync+scalar rings: the SDMA engines
  round-robin per-DESCRIPTOR between rings, so fat (16KB) descriptors on
  one ring delay the other ring's completion sems by ~8us.
- k-outer matmul order (8 interleaved PSUM accumulation groups): the
  Tile scheduler serializes the evictions ~10us late; stalls >5us also
  let HAM re-throttle the PE to 1.2GHz mid-stream.
- Splitting the final store in halves across rings: two ~2us fixed DGE
  latencies serialize behind the serial DVE casts; one per-j store tail
  is optimal.

v7 = v4 with the head resliced so the first j-sweep unblocks after
512KB instead of 768KB: ring order w[0:2], x0c0, w[2:4], w[4:8],
x0c1..7, then the s>=1 superblocks. The j0 sweep starts on w k0-1 +
x0c0 and catches up on w[2:4]/w[4:8] mid-sweep.

Device-side layouts (p = SBUF partition = low 7 bits of the d index,
t = token-in-tile, j = token tile, k = contraction tile, s = super):
  xs [128, NSUP, TPS*KT*128]: xs[p, s, j*1024 + k*128 + t]
                                = x[s*1024 + j*128 + t, k*128 + p]
  ws [128, KT*OPP]:           ws[p, k*512 + c] = W.T[d, c]
  ys [128, NSUP, TPS*OPP]:    ys[p, s, j*512 + c] = y[s*1024 + j*128 + p, c]
"""

import sys

if "/opt/trn_rl_repo" not in sys.path:
    sys.path.insert(0, "/opt/trn_rl_repo")

import numpy as np

VARIANT = "v7"

# Problem constants (hardcoded per harness contract).
E = 8
BS = 64
S = 1024
D = 1024
OPP = 512
P = 128
TOK = (BS // E) * S  # 8192 tokens per expert
KT = D // P          # 8 contraction tiles
TW = 1024            # token-superblock width staged in SBUF
NSUP = TOK // TW     # 8 superblocks
TPS = TW // P        # 8 token tiles (of 128) per superblock
CH = KT * P          # 1024 elements per chunk per partition (j- or k-chunk)

N_WARM = 10          # warmup matmuls bridging the initial DMA window

_programs: dict[str, tuple] = {}


def _build():
    import concourse.bacc as bacc
    import concourse.tile as tile
    from concourse import mybir
    import ml_dtypes

    mm_dt = mybir.dt.bfloat16
    np_in = ml_dtypes.bfloat16

    nc = bacc.Bacc(None, target_bir_lowering=False, debug=False)

    xs = nc.dram_tensor("xs", [P, NSUP, TPS * KT * P], mm_dt, kind="ExternalInput")
    ws = nc.dram_tensor("ws", [P, KT * OPP], mm_dt, kind="ExternalInput")
    ys = nc.dram_tensor("ys", [P, NSUP, TPS * OPP], mm_dt, kind="ExternalOutput")

    with tile.TileContext(nc) as tc:
        with (
            tc.tile_pool(name="wpool", bufs=1) as wpool,
            tc.tile_pool(name="xpool", bufs=4) as xpool,
            tc.tile_pool(name="opool", bufs=3) as opool,
            tc.tile_pool(name="pspool", bufs=8, space="PSUM") as pspool,
        ):
            x0_sb = xpool.tile([P, TPS * KT * P], mm_dt, tag="x")
            w_sb = wpool.tile([P, KT * OPP], mm_dt)
            # All loads on ONE ring (sync), in consumption order. Two-queue
            # "parallel" loading is a trap: the SDMA engines round-robin
            # per-DESCRIPTOR between rings, so a ring with fat (16KB)
            # descriptors starves the completion sems of a ring with thin
            # ones by ~8us (measured, v6). Within one ring, FIFO order IS
            # priority, so slice the head so the first j-sweep unblocks
            # after 512KB (w k0-1 + x0 chunk j0), not 768KB (v4):
            # w[0:2], x0c0, w[2:4], w[4:8], x0c1..7, then the s>=1
            # superblocks. Stores live on the scalar ring.
            nc.sync.dma_start(out=w_sb[:, 0 : 2 * OPP], in_=ws[:, 0 : 2 * OPP])
            nc.sync.dma_start(out=x0_sb[:, 0:CH], in_=xs[:, 0, 0:CH])
            nc.sync.dma_start(
                out=w_sb[:, 2 * OPP : 4 * OPP], in_=ws[:, 2 * OPP : 4 * OPP]
            )
            nc.sync.dma_start(out=w_sb[:, 4 * OPP :], in_=ws[:, 4 * OPP :])
            for j in range(1, TPS):
                nc.sync.dma_start(
                    out=x0_sb[:, j * CH : (j + 1) * CH],
                    in_=xs[:, 0, j * CH : (j + 1) * CH],
                )

            # PE warmup on a zeroed tile: keeps the PE busy from shortly
            # after Tile entry so HAM un-throttles (1.2->2.4GHz) ~3.4us
            # later, right before the real stream starts; the idle gap to
            # the first real matmul stays under the ~5us re-throttle
            # window.
            warm_src = wpool.tile([P, OPP], mm_dt, tag="warm")
            nc.gpsimd.memset(warm_src[:], 0.0)
            warm_ps = pspool.tile([P, OPP], mybir.dt.float32, tag="ps")
            for _ in range(N_WARM):
                nc.tensor.matmul(
                    warm_ps[:], warm_src[:, :P], warm_src[:], start=True, stop=True
                )

            # All superblocks: j-outer steady state (proven clean pipeline;
            # k-outer variants confused the Tile scheduler into serializing
            # the PSUM evictions ~10us late).
            for s in range(NSUP):
                if s == 0:
                    x_sb = x0_sb
                else:
                    x_sb = xpool.tile([P, TPS * KT * P], mm_dt, tag="x")
                    nc.sync.dma_start(out=x_sb[:], in_=xs[:, s, :])
                o_sb = opool.tile([P, TPS * OPP], mm_dt, tag="o")
                last_s = s == NSUP - 1
                for j in range(TPS):
                    ps = pspool.tile([P, OPP], mybir.dt.float32, tag="ps")
                    for k in range(KT):
                        nc.tensor.matmul(
                            ps[:],
                            x_sb[:, j * CH + k * P : j * CH + (k + 1) * P],
                            w_sb[:, k * OPP : (k + 1) * OPP],
                            start=(k == 0),
                            stop=(k == KT - 1),
                        )
                    nc.vector.tensor_copy(
                        o_sb[:, j * OPP : (j + 1) * OPP], ps[:]
                    )
                    if last_s:
                        # Fine-grained tail: store each token tile as soon
                        # as it is evicted.
                        nc.scalar.dma_start(
                            out=ys[:, s, j * OPP : (j + 1) * OPP],
                            in_=o_sb[:, j * OPP : (j + 1) * OPP],
                        )
                if not last_s:
                    nc.scalar.dma_start(out=ys[:, s, :], in_=o_sb[:])

    nc.compile()
    return nc, np_in


def _get_program():
    if VARIANT not in _programs:
        _programs[VARIANT] = _build()
    return _programs[VARIANT]


def kernel(input_, idx_list, W, b, **_ignored):
    from concourse.bass_utils import run_bass_kernel_spmd

    input_ = np.asarray(input_)
    idx = np.asarray(idx_list).astype(np.int64)
    W = np.asarray(W, dtype=np.float32)
    b = np.asarray(b, dtype=np.float32)

    nc, np_in = _get_program()

    in_maps = []
    for e in range(E):
        xg = input_[idx[e]].reshape(TOK, D)
        # xs[p, s, j*1024 + k*128 + t] = x[s*1024 + j*128 + t, k*128 + p]
        xhost = np.ascontiguousarray(
            xg.reshape(NSUP, TPS, P, KT, P).transpose(4, 0, 1, 3, 2)
        ).reshape(P, NSUP, TPS * KT * P)
        # ws[p, k*512 + c] = W[c, k*128 + p]
        whost = np.ascontiguousarray(
            W[e].reshape(OPP, KT, P).transpose(2, 1, 0)
        ).reshape(P, KT * OPP)
        in_maps.append(
            {"xs": xhost.astype(np_in), "ws": whost.astype(np_in)}
        )

    res = run_bass_kernel_spmd(nc, in_maps, core_ids=list(range(E)))

    out = np.zeros((BS, S, E * OPP), dtype=input_.dtype)
    for e in range(E):
        yd = np.asarray(res.results[e]["ys"]).astype(input_.dtype)
        # ys[p, s, j*512 + c] -> y[s*1024 + j*128 + p, c]
        ye = yd.reshape(P, NSUP, TPS, OPP).transpose(1, 2, 0, 3).reshape(
            BS // E, S, OPP
        )
        if b[e].any():
            ye = ye + b[e][None, None, :]
        out[idx[e], :, e * OPP : (e + 1) * OPP] = ye
    return out


# revision 9
# speedup vs baseline: 1.0956x; 1.0228x over previous
"""ColumnParallelLinearWithMoE Trainium2 kernel.

Expert-parallel: expert e -> NeuronCore e. Each core computes
    y_e [8192, 512] = x_e [8192, 1024] @ W_e.T [1024, 512] (+ b_e)
where x_e = input_[idx_list[e]] flattened over (per, seq).

Routing gather/scatter and all layout shuffles happen on the host; the
device does the dense matmul in bf16 (fp32 PSUM accumulation) and stores
bf16. The bias is zero in this module (zero-initialized); if a nonzero
bias ever shows up it is applied on the host during unshard, so the
device pipeline is matmul -> DVE copy (fp32->bf16) -> store.

Timing model (measured on HW):
- exec ~= (last_matmul_ts - 6.0us) + 11.1us: the clock starts at the
  first user instruction (~6us into the trace) and a fixed ~11us tail
  follows the last matmul (final evict+store drain ~3us + a fixed ~8us
  TileContext teardown that clears all 254 semaphores regardless of how
  many the kernel used).
- Body floor is 512 matmuls x 216ns = 110.6us (N=512 moving operand,
  warm PE at 2.4GHz). So the game is: start the warm matmul stream as
  early as possible.
- Head floor: engine preambles end ~7.2us (sync/scalar) and ~7.8us
  (tensor); w is 1MB = 2.8us of HBM transfer, so the k-accumulation of
  the first token tile finishes ~11.5us at best. A 5-matmul PE warmup
  from 7.8us makes HAM un-throttle (1.2->2.4GHz) by ~11.2us.
- DMA issue cost on the HWDGE rings scales with descriptor count
  (~3.5us for a 1024-descriptor 2MB load). All DRAM layouts here are
  host-shuffled so every transfer is contiguous per partition (128
  descriptors, ~0.6us issue).
- PSUM dependency tracking is tile-granular: per-j [128,512] PSUM tiles
  so each eviction waits only on its own 8-matmul accumulation group.
- The last super evicts/stores per token-tile so the post-matmul drain
  stays short.

Measured refinements (second optimization session; traces in
/tmp/trace_{base,v5,v6,v7}):
- Warm matmul cadence is 221.4ns (513/512-col cadence incl NX dispatch
  of the mm+LDWEIGHTS pair); body = 512 x 221.4 = 113.4us, gapless but
  for 11 periodic 52ns hiccups. The measured plateau of this program is
  131.6-133.4us run-to-run (HAM phase + preamble jitter).
- The ~8us tail after the last store is a framework postamble (254-
  semaphore clear storm, ~55 EVENT_SEMAPHOREs per engine in parallel,
  Tensor slowest at ~115ns each) plus 2 barriers; not kernel-
  proportional, not avoidable from kernel code.
- Stream-entry floor: every output needs ALL of w (1MB) in SBUF, DMA
  completion sems lag last-byte by 1.2-3us (16 per-SDMA-engine incs),
  and the first ~1MB of ring data moves at only ~150-300GB/s while the
  SDMA/HBM pipe fills. Hence j0's sweep cannot complete before ~10.7us
  after the exec window opens; this kernel's order (w[0:4], x0c0,
  w[4:8], x0c1..7) achieves that floor.

Dead ends measured on HW - do NOT revisit (v5 142.8us, v6 141.8us,
v7 133.4us):
- Splitting loads across the sync+scalar rings: SDMA engines round-robin
  per-DESCRIPTOR between rings, so fat (16KB) descriptors on one ring
  delay the thin ring's completion sems by ~8us (v6).
- Fine-grained w k-chunks (128KB, 1KB descriptors): ~66GB/s each, clog
  the ring for ~10us (v5).
- k-outer matmul order (8 interleaved PSUM accumulation groups): Tile
  serializes the evictions ~10us late even when CASTs are emitted right
  after each stop-matmul; stalls >5.2us also let HAM re-throttle the PE
  to 1.2GHz mid-stream (v5/v6).
- Splitting the final store into halves on two rings: two ~2us fixed
  DGE latencies serialize behind the serial DVE casts (v5); the v4
  per-j tail is optimal (~2.9us last-mm -> store-complete).
- fp8: double-pump is 2x theoretical (~1.5x measured); e4m3 on both
  operands gives rel_fro ~3.3e-2 > the 2e-2 gate, and the 3-matmul
  precision split costs 1.5x -> never beats bf16. bf16 PSUM accumulate
  (1024-wide moving) is TRN3-only; TRN2 matmul output must be fp32.
- LDWEIGHTS amortization (w-stationary, shared stationary across token
  halves) forces 8 simultaneously-open PSUM groups per superblock ->
  bunched evictions collide with the next superblock's first octet.

Device-side layouts (p = SBUF partition = low 7 bits of the d/contraction
index, t = token-in-tile, j = token tile, k = contraction tile, s = super):
  xs [128, NSUP, TPS*KT*128]: xs[p, s, j*1024 + k*128 + t] = x[token, d]
  ws [128, KT*OPP]:           ws[p, k*512 + c] = W.T[d, c]
  ys [128, NSUP, TPS*OPP]:    ys[p, s, j*512 + c] = y[token, c]
with token = s*1024 + j*128 + t(=p for ys), d = k*128 + p.
"""

import sys

if "/opt/trn_rl_repo" not in sys.path:
    sys.path.insert(0, "/opt/trn_rl_repo")

import numpy as np

# Problem constants (hardcoded per harness contract).
E = 8
BS = 64
S = 1024
D = 1024
OPP = 512
P = 128
TOK = (BS // E) * S  # 8192 tokens per expert
KT = D // P          # 8 contraction tiles
TW = 1024            # token-superblock width staged in SBUF
NSUP = TOK // TW     # 8 superblocks
TPS = TW // P        # 8 token tiles (of 128) per superblock

N_WARM = 10          # warmup matmuls bridging the initial DMA window

_programs: dict[str, tuple] = {}


def _build():
    import concourse.bacc as bacc
    import concourse.tile as tile
    from concourse import mybir
    import ml_dtypes

    mm_dt = mybir.dt.bfloat16
    np_in = ml_dtypes.bfloat16

    nc = bacc.Bacc(None, target_bir_lowering=False, debug=False)

    xs = nc.dram_tensor("xs", [P, NSUP, TPS * KT * P], mm_dt, kind="ExternalInput")
    ws = nc.dram_tensor("ws", [P, KT * OPP], mm_dt, kind="ExternalInput")
    ys = nc.dram_tensor("ys", [P, NSUP, TPS * OPP], mm_dt, kind="ExternalOutput")

    with tile.TileContext(nc) as tc:
        with (
            tc.tile_pool(name="wpool", bufs=1) as wpool,
            tc.tile_pool(name="xpool", bufs=4) as xpool,
            tc.tile_pool(name="opool", bufs=3) as opool,
            tc.tile_pool(name="pspool", bufs=8, space="PSUM") as pspool,
        ):
            # Ramp: x token-chunks on the sync ring, w k-chunks on the
            # scalar ring, issuing in parallel from the moment each engine's
            # preamble ends. Every transfer is per-partition contiguous.
            x0_sb = xpool.tile([P, TPS * KT * P], mm_dt, tag="x")
            w_sb = wpool.tile([P, KT * OPP], mm_dt)
            CH = KT * P  # 1024 elements per 128-token chunk per partition
            # All loads go on ONE ring (sync) in consumption order: the two
            # HWDGE rings round-robin on the shared SDMA engines, so a
            # second ring gives fair interleaving, not priority -- but
            # within a ring, FIFO order IS priority. Consume order:
            # w[0:4] (first half of the k-sweep), x0 chunk 0, w[4:8],
            # then the remaining token chunks; superblock loads queue
            # behind in the s-loop. Stores live on the scalar ring.
            nc.sync.dma_start(out=w_sb[:, 0 : 4 * OPP], in_=ws[:, 0 : 4 * OPP])
            nc.sync.dma_start(out=x0_sb[:, 0:CH], in_=xs[:, 0, 0:CH])
            nc.sync.dma_start(out=w_sb[:, 4 * OPP :], in_=ws[:, 4 * OPP :])
            for j in range(1, TPS):
                nc.sync.dma_start(
                    out=x0_sb[:, j * CH : (j + 1) * CH],
                    in_=xs[:, 0, j * CH : (j + 1) * CH],
                )

            # PE warmup on a zeroed tile: keeps the PE busy from ~7.8us
            # (end of the tensor-engine preamble) so HAM un-throttles by
            # ~11.2us, right when the real stream reaches full rate.
            warm_src = wpool.tile([P, OPP], mm_dt, tag="warm")
            nc.gpsimd.memset(warm_src[:], 0.0)
            warm_ps = pspool.tile([P, OPP], mybir.dt.float32, tag="ps")
            for _ in range(N_WARM):
                nc.tensor.matmul(
                    warm_ps[:], warm_src[:, :P], warm_src[:], start=True, stop=True
                )

            for s in range(NSUP):
                if s == 0:
                    x_sb = x0_sb
                else:
                    x_sb = xpool.tile([P, TPS * KT * P], mm_dt, tag="x")
                    nc.sync.dma_start(out=x_sb[:], in_=xs[:, s, :])
                o_sb = opool.tile([P, TPS * OPP], mm_dt, tag="o")
                last_s = s == NSUP - 1
                for j in range(TPS):
                    ps = pspool.tile([P, OPP], mybir.dt.float32, tag="ps")
                    for k in range(KT):
                        nc.tensor.matmul(
                            ps[:],
                            x_sb[:, j * CH + k * P : j * CH + (k + 1) * P],
                            w_sb[:, k * OPP : (k + 1) * OPP],
                            start=(k == 0),
                            stop=(k == KT - 1),
                        )
                    nc.vector.tensor_copy(o_sb[:, j * OPP : (j + 1) * OPP], ps[:])
                    if last_s:
                        # Fine-grained tail: store each token tile as soon
                        # as it is evicted.
                        nc.scalar.dma_start(
                            out=ys[:, s, j * OPP : (j + 1) * OPP],
                            in_=o_sb[:, j * OPP : (j + 1) * OPP],
                        )
                if not last_s:
                    nc.scalar.dma_start(out=ys[:, s, :], in_=o_sb[:])

    nc.compile()
    return nc, np_in


def _get_program():
    if "v4" not in _programs:
        _programs["v4"] = _build()
    return _programs["v4"]


def kernel(input_, idx_list, W, b, **_ignored):
    from concourse.bass_utils import run_bass_kernel_spmd

    input_ = np.asarray(input_)
    idx = np.asarray(idx_list).astype(np.int64)
    W = np.asarray(W, dtype=np.float32)
    b = np.asarray(b, dtype=np.float32)

    nc, np_in = _get_program()

    in_maps = []
    for e in range(E):
        xg = input_[idx[e]].reshape(TOK, D)
        # xs[p, s, j*1024 + k*128 + t] = x[s*1024 + j*128 + t, k*128 + p]
        xhost = np.ascontiguousarray(
            xg.reshape(NSUP, TPS, P, KT, P).transpose(4, 0, 1, 3, 2)
        ).reshape(P, NSUP, TPS * KT * P)
        # ws[p, k*512 + c] = W[c, k*128 + p]
        whost = np.ascontiguousarray(
            W[e].reshape(OPP, KT, P).transpose(2, 1, 0)
        ).reshape(P, KT * OPP)
        in_maps.append(
            {"xs": xhost.astype(np_in), "ws": whost.astype(np_in)}
        )

    res = run_bass_kernel_spmd(nc, in_maps, core_ids=list(range(E)))

    out = np.zeros((BS, S, E * OPP), dtype=input_.dtype)
    for e in range(E):
        yd = np.asarray(res.results[e]["ys"]).astype(input_.dtype)
        # ys[p, s, j*512 + c] -> y[s*1024 + j*128 + p, c]
        ye = yd.reshape(P, NSUP, TPS, OPP).transpose(1, 2, 0, 3).reshape(
            BS // E, S, OPP
        )
        if b[e].any():
            ye = ye + b[e][None, None, :]
        out[idx[e], :, e * OPP : (e + 1) * OPP] = ye
    return out
